# revision 1
# baseline (speedup 1.0000x reference)
"""Trainium2 Bass kernel v3 for nn_EntityBranch (adapter -> BiLSTM -> proto/cdist -> CRF loss).

Sharding: time+direction parallel. Core c: dir d=c&1 (0 fwd, 1 bwd), window
w=c>>1 (128 global timesteps), all 32 items. The LSTM scan is parallelized
over time via a WU-step redundant warmup (forget-gate contraction makes the
handoff error ~1e-6 at the loss). Items are permuted into language order so
adapter weights stay PE-stationary across item blocks.

Per-core pipeline:
  A. adapter (redundant per window): yT = W1c.T @ xT per lang block (W1
     mean-centered on host so LN mean is exactly 0); var via ones-matmul of
     y^2; z = relu(y) on gpsimd; xp = WF.T @ z; LN rstd folded into the
     psum->xpT copy (post-scale). All weights fp8-e3m4. Emitted interleaved
     with B steps so adapter matmuls fill the LSTM's dependency gaps.
  B. one-dir LSTM, T=WU+128 steps, 32 items: per step one identity-matmul
     accumulates xp into PSUM (kills the DVE gpre add), 16 fp8 Whh matmuls,
     tanh from PSUM, all-tanh gate algebra split DVE/gpsimd/ACT.
     h written at slot WU+rev7(s-WU) (bit-reversed window time).
  C. pairwise AllGather (2-core groups) of window h; both cores of a pair
     build [hf|hb] (bwd source read sigma-flipped), then ef/proto/emission
     distances for 128 slots x 32 items.
  D. per-window CRF: N_t = trans + em_t, log-matmul tree over 128 slots
     (5 chunk levels + 2 partition levels), numerator partials via one-hot
     algebra. Outputs per item: 5x5 window product, numerator partial, em0,
     pl vector. Host combines the 4 windows per item (tiny 5x5 log-matmuls)
     + start/end/boundary-transition terms.
"""

import sys

sys.path.insert(0, "/opt/trn_rl_repo")

import numpy as np
import ml_dtypes

import concourse.bass as bass
import concourse.bacc as bacc
import concourse.mybir as mybir
import concourse.tile as tile
from concourse.bass_utils import run_bass_kernel_spmd

F8 = mybir.dt.float8e3
F16 = mybir.dt.float16
F32 = mybir.dt.float32
AF = mybir.ActivationFunctionType
OP = mybir.AluOpType
NP8 = ml_dtypes.float8_e3m4
NP16 = np.float16

# --- problem constants ---
B, S, H = 32, 512, 768
HL = 256
EF, PD, L = 256, 256 // 2, 5  # PD=128
NCORES = 8
NLANG = 5
PROTO_W = 0.5
EPS = 1e-5
NEG = -1.0e9

# --- sharding constants ---
W = 128          # window steps per core
WU = 16          # warmup steps
T = W + WU       # local steps
NCH = 4          # adapter chunks
CH = T // NCH    # steps per adapter chunk
IBMAX = 8        # max items per adapter lang-block (psum budget)
ROWS = W * B     # phase-C rows per core (128 slots x 32 items)
NRC = ROWS // 128            # 32 chunks of 128 rows
SBn = W // 4                 # 32 demc chunks (sigma>>2)


def _rev(t: int, nbits: int) -> int:
    r = 0
    for i in range(nbits):
        r |= ((t >> i) & 1) << (nbits - 1 - i)
    return r


def _ap(ap, dims):
    """Custom free-dim AP on same tensor/offset: dims = [[step, count], ...]."""
    return bass.AP(tensor=ap.tensor, offset=ap.offset, ap=[list(ap.ap[0])] + dims)


def _slot(s: int) -> int:
    """hT slot for local step s."""
    return s if s < WU else WU + _rev(s - WU, 7)


# ===========================================================================
# device program
# ===========================================================================


def build_kernel(nc: bass.Bass, blocks, markers=None):
    """blocks: list of (lang, j0, cnt) adapter item-blocks (j = permuted item
    index; contiguous since items are lang-sorted)."""

    def mark(label):
        if markers is not None:
            markers[label] = nc.next_id()

    P = {}

    def par(name, shape, dtype=F16):
        P[name] = nc.declare_dram_parameter(name, list(shape), dtype, isOutput=False)
        return P[name]

    XT = par("XT", [128, NCH, 6, CH * B], F8)
    W1T = par("W1T", [128, NLANG, 6, 6, 128], F8)
    WFh = par("WFh", [128, NLANG, 6, 8, 128], F8)
    WHH = par("WHH", [128, 2, 8, 128], F8)
    IDN8 = par("IDN8", [128, 128], F8)
    ONESW = par("ONESW", [128, 128], F16)
    PJh = par("PJh", [128, 2, 2, EF])
    PW1h = par("PW1h", [128, 2, PD])
    PW2h = par("PW2h", [128, PD])
    SEFT = par("SEFT", [128, 2, L])
    PROT = par("PROT", [128, L])
    IDN = par("IDN", [128, 128])
    SEL32 = par("SEL32", [128, B], F32)
    ONES1 = par("ONES1", [128, 1], F32)
    TRR = par("TRR", [128, L * L], F32)
    IOTA = par("IOTA", [128, L], F32)
    LOGID = par("LOGID", [B, L * L], F32)
    LABC = par("LABC", [128, SBn], F32)
    LABN = par("LABN", [128, SBn], F32)
    TINV2 = par("TINV2", [128, 1], F32)
    OUT = nc.declare_dram_parameter("OUT", [B, 32], F32, isOutput=True)

    with tile.TileContext(nc) as tc, \
            tc.tile_pool(name="persist", bufs=1) as pp, \
            tc.tile_pool(name="dram", bufs=1, space="DRAM") as dram:
        # ------------- persistent tiles -------------
        whh = pp.tile([128, 2, 8, 128], F8, tag="whh")
        idn8 = pp.tile([128, 128], F8, tag="idn8")
        onesw = pp.tile([128, 128], F16, tag="onesw")
        idn = pp.tile([128, 128], F16, tag="idn")
        cst = pp.tile([128, 30], F32, tag="cst")
        sel32 = pp.tile([128, B], F32, tag="sel32")
        ones1 = pp.tile([128, 1], F32, tag="ones1")
        labc = pp.tile([128, SBn], F32, tag="labc")
        labn = pp.tile([128, SBn], F32, tag="labn")
        zeroH = pp.tile([128, B], F8, tag="zeroH")
        zeroC = pp.tile([128, 2 * B], F32, tag="zeroC")
        tinv2 = pp.tile([128, 1], F32, tag="tinv2")
        epst = pp.tile([128, 1], F32, tag="epst")
        onesr = pp.tile([1, 128], F32, tag="onesr")
        demc = pp.tile([128, SBn, L], F32, tag="demc")

        nc.sync.dma_start(out=whh[:], in_=WHH[:])
        nc.sync.dma_start(out=idn8[:], in_=IDN8[:])
        nc.sync.dma_start(out=onesw[:], in_=ONESW[:])
        nc.sync.dma_start(out=idn[:], in_=IDN[:])
        nc.sync.dma_start(out=cst[:, 0:25], in_=TRR[:])
        nc.sync.dma_start(out=cst[:, 25:30], in_=IOTA[:])
        nc.sync.dma_start(out=sel32[:], in_=SEL32[:])
        nc.sync.dma_start(out=ones1[:], in_=ONES1[:])
        nc.sync.dma_start(out=labc[:], in_=LABC[:])
        nc.sync.dma_start(out=labn[:], in_=LABN[:])
        nc.sync.dma_start(out=tinv2[:], in_=TINV2[:])
        nc.vector.memset(zeroH[:], 0.0)
        nc.vector.memset(zeroC[:], 0.0)
        nc.vector.memset(epst[:], EPS)
        nc.vector.memset(onesr[:], 1.0)

        trans_r = cst[:, 0:25]
        iota_r = cst[:, 25:30]

        # ================= Phase A thunks (interleaved with B) =============
        a_thunks = []  # list of closures per chunk, [ch][i]

        with (
            tc.tile_pool(name="abig", bufs=1) as abig,
            tc.tile_pool(name="axs", bufs=2) as axs,
            tc.tile_pool(name="ast", bufs=2) as ast,
            tc.tile_pool(name="asm", bufs=3) as asm,
            tc.tile_pool(name="psA", bufs=2, space="PSUM") as psA,
            tc.tile_pool(name="psS", bufs=2, space="PSUM") as psS,
            tc.tile_pool(name="psX", bufs=2, space="PSUM") as psX,
            tc.tile_pool(name="psB", bufs=2, space="PSUM") as psB,
            tc.tile_pool(name="gp", bufs=3) as gp,
            tc.tile_pool(name="stp", bufs=3) as stp,
        ):
            # xpT: [p(gate in cb-chunk), cb, t, item]; hT: [p, hk, item, slot]
            xpT = abig.tile([128, 8, T, B], F8, tag="xpT")
            hT = abig.tile([128, 2, B, T], F8, tag="hT")
            W1T_t = abig.tile([128, NLANG, 6, 6, 128], F8, tag="wst")
            WFh_t = abig.tile([128, NLANG, 6, 8, 128], F8, tag="wft")
            nc.sync.dma_start(out=W1T_t[:], in_=W1T[:])
            nc.sync.dma_start(out=WFh_t[:], in_=WFh[:])
            for ch in range(NCH):
                thunks = []
                state = {}

                def dma_x(ch=ch, state=state):
                    xts = axs.tile([128, 6, CH * B], F8, tag="xts", name="xts")
                    nc.sync.dma_start(out=xts[:], in_=XT[:, ch])
                    state["xts"] = xts

                thunks.append(dma_x)

                for (lg, j0, cnt) in blocks:
                    R = cnt * CH
                    c0 = j0 * CH
                    bst = {}

                    def alloc_blk(R=R, bst=bst):
                        bst["ysq"] = ast.tile([128, 6, IBMAX * CH], F16, tag="ysq", name="ysq")
                        bst["z16"] = ast.tile([128, 6, IBMAX * CH], F8, tag="z16", name="z16")

                    thunks.append(alloc_blk)

                    for ko in range(6):
                        def w1_ko(lg=lg, c0=c0, R=R, ko=ko, bst=bst, state=state):
                            psy = psA.tile([128, IBMAX * CH], F32, tag="psy", name="psy")
                            for ki in range(6):
                                nc.tensor.matmul(
                                    psy[:, 0:R],
                                    W1T_t[:, lg, ki, ko, :],
                                    state["xts"][:, ki, c0:c0 + R],
                                    start=(ki == 0),
                                    stop=(ki == 5),
                                )
                            nc.scalar.activation(
                                bst["ysq"][:, ko, 0:R], psy[:, 0:R], AF.Square
                            )
                            # gpsimd cannot read PSUM on HW; relu on DVE
                            nc.vector.tensor_scalar_max(
                                bst["z16"][:, ko, 0:R], psy[:, 0:R], 0.0
                            )

                        thunks.append(w1_ko)

                    def stats_blk(R=R, bst=bst):
                        pss = psS.tile([128, IBMAX * CH], F32, tag="pss", name="pss")
                        for k in range(6):
                            nc.tensor.matmul(
                                pss[:, 0:R],
                                onesw[:],
                                bst["ysq"][:, k, 0:R],
                                start=(k == 0),
                                stop=(k == 5),
                            )
                        sd = asm.tile([128, IBMAX * CH], F32, tag="sd", name="sd")
                        rr = asm.tile([128, IBMAX * CH], F32, tag="rr", name="rr")
                        nc.scalar.activation(
                            sd[:, 0:R], pss[:, 0:R], AF.Sqrt,
                            bias=epst[:], scale=1.0 / H,
                        )
                        nc.vector.reciprocal(rr[:, 0:R], sd[:, 0:R])
                        bst["rr"] = rr

                    thunks.append(stats_blk)

                    for cb in range(8):
                        def xp_cb(lg=lg, j0=j0, cnt=cnt, R=R, ch=ch, cb=cb, bst=bst):
                            psx = psX.tile([128, IBMAX * CH], F32, tag="psx", name="psx")
                            for k in range(6):
                                nc.tensor.matmul(
                                    psx[:, 0:R],
                                    WFh_t[:, lg, k, cb, :],
                                    bst["z16"][:, k, 0:R],
                                    start=(k == 0),
                                    stop=(k == 5),
                                )
                            # out: xpT[:, cb, ch*CH + tc, j0 + j]; psx cols (j, tc)
                            out = _ap(
                                xpT[:, cb, ch * CH, j0],
                                [[1, cnt], [B, CH]],
                            )
                            nc.vector.tensor_tensor(
                                out=out, in0=psx[:, 0:R], in1=bst["rr"][:, 0:R],
                                op=OP.mult,
                            )

                        thunks.append(xp_cb)

                a_thunks.append(thunks)

            # ============ interleaved emission: A chunks + B steps =========
            # chunk 0 fully first
            mark("a0_start")
            for th_ in a_thunks[0]:
                th_()
            mark("a0_done")
            pend = [th_ for chth in a_thunks[1:] for th_ in chth]
            npend = len(pend)
            pi = 0

            c_prev = zeroC
            for s in range(T):
                mark(f"b{s}")
                ps = psB.tile([128, 8 * B], F32, tag="pstep")
                nc.tensor.matmul(
                    ps[:], idn8[:], xpT[:, :, s, :], start=True, stop=False,
                    skip_group_check=True,
                )
                for k in range(2):
                    if s == 0:
                        rhs = zeroH[:]
                    else:
                        rhs = _ap(hT[:, k, 0, _slot(s - 1)], [[T, B]])
                    for cb in range(8):
                        nc.tensor.matmul(
                            ps[:, cb * B:(cb + 1) * B],
                            whh[:, k, cb, :],
                            rhs,
                            start=False,
                            stop=(k == 1),
                            skip_group_check=True,
                        )
                # gate block order i,f,g,o: tanh of i,f,g first so the
                # c-update starts while the o-tanh runs
                th = gp.tile([128, 8 * B], F16, tag="th")
                nc.scalar.activation(th[:, 0:6 * B], ps[:, 0:6 * B], AF.Tanh)
                aa = stp.tile([128, 2 * B], F32, tag="aa")
                bb = stp.tile([128, 2 * B], F32, tag="bb")
                cn = stp.tile([128, 2 * B], F32, tag="cn")
                tcc = stp.tile([128, 2 * B], F16, tag="tcc")
                nc.vector.scalar_tensor_tensor(
                    aa[:], th[:, 2 * B:4 * B], 1.0, c_prev[:], op0=OP.add, op1=OP.mult
                )
                nc.vector.scalar_tensor_tensor(
                    bb[:], th[:, 0:2 * B], 1.0, th[:, 4 * B:6 * B],
                    op0=OP.add, op1=OP.mult,
                )
                nc.vector.scalar_tensor_tensor(
                    cn[:], aa[:], 0.5, bb[:], op0=OP.mult, op1=OP.add
                )
                nc.scalar.activation(th[:, 6 * B:8 * B], ps[:, 6 * B:8 * B], AF.Tanh)
                nc.scalar.activation(tcc[:], cn[:], AF.Tanh, scale=0.5)
                hout = _ap(hT[:, 0, 0, _slot(s)], [[B * T, 2], [T, B]])
                nc.vector.scalar_tensor_tensor(
                    hout, th[:, 6 * B:8 * B], 1.0, tcc[:], op0=OP.add, op1=OP.mult
                )
                c_prev = cn

                # pace A thunks: chunk ch+1 done by step (ch+1)*CH
                due = min(npend, ((s + 1) * npend) // (3 * CH)) if s < 3 * CH else npend
                while pi < due:
                    pend[pi]()
                    pi += 1
            while pi < npend:
                pend[pi]()
                pi += 1

            mark("b_done")
            # stage window h to DRAM for the pair exchange (fp8, direct DMA)
            HTD = dram.tile([128, 2, B, W], F8, tag="HTD")
            nc.sync.dma_start(out=HTD[:], in_=hT[:, :, :, WU:T])

        # ================= exchange: pairwise AllGather of window h ========
        mark("exch_start")
        HTG = dram.tile([2, 128, 2, B, W], F8, tag="HTG")
        nc.gpsimd.collective_compute(
            "AllGather",
            OP.bypass,
            replica_groups=[[0, 1], [2, 3], [4, 5], [6, 7]],
            ins=[HTD.opt()],
            outs=[HTG.opt()],
        )

        # ================= Phase C: features / emissions / support =========
        with (
            tc.tile_pool(name="cw", bufs=1) as cw,
            tc.tile_pool(name="cbig", bufs=1) as cbig,
            tc.tile_pool(name="psC", bufs=2, space="PSUM") as psC,
            tc.tile_pool(name="psC2", bufs=1, space="PSUM") as psC2,
            tc.tile_pool(name="cs", bufs=4) as cs,
        ):
            hTG = cbig.tile([128, 2, 2, B, W], F8, tag="hTG")
            nc.sync.dma_start(out=hTG[:], in_=HTG[:])
            # hTC[p, sigma, col]: col = src*64 + hk*32 + j (src0=fwd)
            hTC = cbig.tile([128, W, 128], F16, tag="hTC")
            for src in range(2):
                for hk in range(2):
                    if src == 0:
                        in_ = _ap(hTG[:, 0, hk, 0, 0], [[1, W], [W, B]])
                    else:
                        in_ = _ap(hTG[:, 1, hk, 0, W - 1], [[-1, W], [W, B]])
                    out = _ap(hTC[:, 0, src * 64 + hk * B], [[128, W], [1, B]])
                    if (src + hk) % 2 == 0:
                        nc.vector.tensor_copy(out, in_)
                    else:
                        nc.scalar.copy(out, in_)

            pj = cw.tile([128, 2, 2, EF], F16, tag="pj")
            pw1 = cw.tile([128, 2, PD], F16, tag="pw1")
            pw2 = cw.tile([128, PD], F16, tag="pw2")
            seft = cw.tile([128, 2, L], F16, tag="seft")
            prot = cw.tile([128, L], F16, tag="prot")
            nc.sync.dma_start(out=pj[:], in_=PJh[:])
            nc.sync.dma_start(out=pw1[:], in_=PW1h[:])
            nc.sync.dma_start(out=pw2[:], in_=PW2h[:])
            nc.sync.dma_start(out=seft[:], in_=SEFT[:])
            nc.sync.dma_start(out=prot[:], in_=PROT[:])

            # ---- support branch ----
            ps5 = psC2.tile([L, PD], F32, tag="ps")
            for k in range(2):
                nc.tensor.matmul(
                    ps5[:], seft[:, k, :], pw1[:, k, :], start=(k == 0), stop=(k == 1)
                )
            stat5 = cs.tile([L, 6], F32, tag="stat5")
            mv5 = cs.tile([L, 2], F32, tag="mv5")
            nc.vector.bn_stats(out=stat5[:], in_=ps5[:])
            nc.vector.bn_aggr(out=mv5[:], in_=stat5[:])
            sd5 = cs.tile([L, 1], F32, tag="sd5")
            rr5 = cs.tile([L, 1], F32, tag="rr5")
            nm5_ = cs.tile([L, 1], F32, tag="nm5_")
            nc.scalar.activation(sd5[:], mv5[:, 1:2], AF.Sqrt, bias=epst[0:L, :])
            nc.vector.reciprocal(rr5[:], sd5[:])
            nc.vector.scalar_tensor_tensor(
                nm5_[:], mv5[:, 0:1], -1.0, rr5[:], op0=OP.mult, op1=OP.mult
            )
            h1s = cs.tile([L, PD], F16, tag="h1s")
            nc.scalar.activation(h1s[:], ps5[:], AF.Relu, bias=nm5_[:], scale=rr5[:])
            psT5 = psC2.tile([128, L], F16, tag="ps")
            nc.tensor.transpose(psT5[:], h1s[:], idn[0:L, 0:L])
            h1sT = cs.tile([128, L], F16, tag="h1sT")
            nc.scalar.copy(h1sT[:], psT5[:])
            psp = psC2.tile([L, PD], F32, tag="ps")
            nc.tensor.matmul(psp[:], h1sT[:], pw2[:], start=True, stop=True)
            sprow = cs.tile([L, PD], F16, tag="sprow")
            nc.scalar.copy(sprow[:], psp[:])
            scr5 = cs.tile([L, PD], F16, tag="scr5")
            sp2r = cs.tile([L, 1], F32, tag="sp2r")
            nc.scalar.activation(scr5[:], psp[:], AF.Square, accum_out=sp2r[:])
            psT5b = psC2.tile([128, L], F16, tag="ps")
            nc.tensor.transpose(psT5b[:], sprow[:], idn[0:L, 0:L])
            spT = cs.tile([128, L], F16, tag="spT")
            nc.scalar.copy(spT[:], psT5b[:])
            sq128 = cs.tile([128, L], F32, tag="sq128")
            nc.vector.tensor_tensor(out=sq128[:], in0=spT[:], in1=spT[:], op=OP.mult)
            psv = psC2.tile([1, L], F32, tag="ps")
            nc.tensor.matmul(psv[:], ones1[:], sq128[:], start=True, stop=True)
            sp2v = cs.tile([1, L], F32, tag="sp2v")
            nc.vector.tensor_copy(sp2v[:], psv[:])
            psrep = psC2.tile([128, L], F32, tag="ps")
            nc.tensor.matmul(psrep[:], onesr[:], sp2v[:], start=True, stop=True)
            sp2rep = cs.tile([128, L], F32, tag="sp2rep")
            nc.vector.tensor_copy(sp2rep[:], psrep[:])


            efT = cbig.tile([128, 2, ROWS], F16, tag="efT")
            h1z = cbig.tile([128, ROWS], F16, tag="h1z")   # relu(yc).T [PD, rows]
            qT = cbig.tile([128, ROWS], F16, tag="qT")
            rrC = cbig.tile([128, ROWS], F32, tag="rrC")   # LN rstd per row (bcast)
            q2sb = cbig.tile([1, ROWS], F32, tag="q2sb")
            q2row = cbig.tile([128, NRC], F32, tag="q2row")

            BLK = 512
            SLB = BLK // B  # 16 slots per block
            nnc = ROWS // BLK  # 8
            for e in range(2):
                for n in range(nnc):
                    pse = psC.tile([128, BLK], F32, tag="ps")
                    first = True
                    for src in range(2):
                        for hk in range(2):
                            nc.tensor.matmul(
                                pse[:],
                                pj[:, src, hk, e * 128:(e + 1) * 128],
                                hTC[:, n * SLB:(n + 1) * SLB,
                                    src * 64 + hk * B:src * 64 + hk * B + B],
                                start=first,
                                stop=(src == 1 and hk == 1),
                            )
                            first = False
                    nc.scalar.copy(efT[:, e, n * BLK:(n + 1) * BLK], pse[:])

            # h1 transposed: y.T = pW1c.T @ efT (pW1 mean-centered on host so
            # LN mean is 0); var via ones-matmul of y^2; rstd folded into the
            # qT copy (relu commutes with the positive scale).
            for n in range(nnc):
                sl = slice(n * BLK, (n + 1) * BLK)
                psh = psC.tile([128, BLK], F32, tag="psh")
                for e in range(2):
                    nc.tensor.matmul(
                        psh[:], pw1[:, e, :], efT[:, e, sl],
                        start=(e == 0), stop=(e == 1),
                    )
                ysq1 = cs.tile([128, BLK], F16, tag="ysq1")
                nc.scalar.activation(ysq1[:], psh[:], AF.Square)
                pss1 = psC2.tile([128, BLK], F32, tag="pstat")
                nc.tensor.matmul(pss1[:], onesw[:], ysq1[:], start=True, stop=True)
                sdc = cs.tile([128, BLK], F32, tag="sdc")
                nc.scalar.activation(
                    sdc[:], pss1[:], AF.Sqrt, bias=epst[:], scale=1.0 / PD
                )
                nc.vector.reciprocal(rrC[:, sl], sdc[:])
                nc.vector.tensor_scalar_max(h1z[:, sl], psh[:], 0.0)

            for n in range(nnc):
                sl = slice(n * BLK, (n + 1) * BLK)
                psq = psC.tile([128, BLK], F32, tag="psq")
                nc.tensor.matmul(
                    psq[:], pw2[:], h1z[:, sl], start=True, stop=True,
                )
                nc.vector.tensor_tensor(
                    out=qT[:, sl], in0=psq[:], in1=rrC[:, sl], op=OP.mult
                )
                qsq = cs.tile([128, BLK], F16, tag="qsq")
                nc.scalar.activation(qsq[:], qT[:, sl], AF.Square)
                psn2 = psC2.tile([128, BLK], F32, tag="pstat")
                nc.tensor.matmul(psn2[:], onesw[:], qsq[:], start=True, stop=True)
                nc.scalar.copy(q2sb[0:1, sl], psn2[0:1, :])
            for rc in range(NRC):
                nc.sync.dma_start(
                    out=q2row[:, rc:rc + 1],
                    in_=q2sb[0:1, rc * 128:(rc + 1) * 128],
                )

            # ---- emissions distances, batched over all row chunks ----
            psg = psC2.tile([128, NRC, L], F32, tag="ps")
            for rc in range(NRC):
                nc.tensor.matmul(
                    psg[:, rc, :], qT[:, rc * 128:(rc + 1) * 128], spT[:],
                    start=True, stop=True,
                )
            d2a = cs.tile([128, NRC, L], F32, tag="d2a")
            nc.vector.scalar_tensor_tensor(
                d2a[:], psg[:], -2.0,
                _ap(q2row[:, 0:1], [[1, NRC], [0, L]]),
                op0=OP.mult, op1=OP.add,
            )
            nc.vector.tensor_tensor(
                out=d2a[:], in0=d2a[:],
                in1=_ap(sp2rep[:, 0:1], [[0, NRC], [1, L]]), op=OP.add,
            )
            nc.vector.tensor_scalar_max(d2a[:], d2a[:], 0.0)
            nc.scalar.activation(demc[:], d2a[:], AF.Sqrt)

            # ---- prototype logits / pl vector ----
            pslg = psC2.tile([L, L], F32, tag="ps")
            nc.tensor.matmul(pslg[:], spT[:], prot[:], start=True, stop=True)
            pr2 = cs.tile([128, L], F32, tag="pr2")
            nc.vector.tensor_tensor(out=pr2[:], in0=prot[:], in1=prot[:], op=OP.mult)
            psv2 = psC2.tile([1, L], F32, tag="ps")
            nc.tensor.matmul(psv2[:], ones1[:], pr2[:], start=True, stop=True)
            pr2v = cs.tile([1, L], F32, tag="pr2v")
            nc.vector.tensor_copy(pr2v[:], psv2[:])
            psrep2 = psC2.tile([L, L], F32, tag="ps")
            nc.tensor.matmul(psrep2[:], onesr[:, 0:L], pr2v[:], start=True, stop=True)
            pr2rep = cs.tile([L, L], F32, tag="pr2rep")
            nc.vector.tensor_copy(pr2rep[:], psrep2[:])
            dl2 = cs.tile([L, L], F32, tag="dl2")
            nc.vector.scalar_tensor_tensor(
                dl2[:], pslg[:], -2.0, _ap(sp2r[:], [[0, L]]), op0=OP.mult, op1=OP.add
            )
            nc.vector.tensor_tensor(out=dl2[:], in0=dl2[:], in1=pr2rep[:], op=OP.add)
            nc.vector.tensor_scalar_max(dl2[:], dl2[:], 0.0)
            dlg = cs.tile([L, L], F32, tag="dlg")
            nc.scalar.activation(dlg[:], dl2[:], AF.Sqrt, scale=tinv2[0:L, :])
            lg = cs.tile([L, L], F32, tag="lg")
            nc.vector.tensor_scalar_mul(lg[:], dlg[:], -1.0)
            m5 = cs.tile([L, 1], F32, tag="m5")
            nc.vector.reduce_max(out=m5[:], in_=lg[:], axis=mybir.AxisListType.X)
            nmm5 = cs.tile([L, 1], F32, tag="nmm5")
            nc.vector.tensor_scalar_mul(nmm5[:], m5[:], -1.0)
            scrl = cs.tile([L, L], F32, tag="scrl")
            se5 = cs.tile([L, 1], F32, tag="se5")
            nc.scalar.activation(scrl[:], lg[:], AF.Exp, bias=nmm5[:], accum_out=se5[:])
            ln5 = cs.tile([L, 1], F32, tag="ln5")
            nc.scalar.activation(ln5[:], se5[:], AF.Ln)
            lse5 = cs.tile([L, 1], F32, tag="lse5")
            nc.vector.tensor_tensor(out=lse5[:], in0=ln5[:], in1=m5[:], op=OP.add)
            dgm = cs.tile([L, L], F32, tag="dgm")
            nc.vector.tensor_tensor(out=dgm[:], in0=lg[:], in1=idn[0:L, 0:L], op=OP.mult)
            dg5 = cs.tile([L, 1], F32, tag="dg5")
            nc.vector.reduce_sum(out=dg5[:], in_=dgm[:], axis=mybir.AxisListType.X)
            plv = cs.tile([L, 1], F32, tag="plv")
            nc.vector.tensor_tensor(out=plv[:], in0=lse5[:], in1=dg5[:], op=OP.subtract)
            nc.sync.dma_start(out=OUT[0:L, 31:32], in_=plv[:])
            # em0 raw distances at u=0 (host negates)
            nc.sync.dma_start(out=OUT[:, 26:31], in_=demc[0:B, 0, :])

            mark("c_done")
            # ============ Phase D: CRF window tree ============
            with (
                tc.tile_pool(name="crf", bufs=2) as crf,
                tc.tile_pool(name="crs", bufs=2) as crs,
            ):
                ntile = crf.tile([128, SBn, 25], F32, tag="ntile")
                for rc in range(SBn):
                    nc.vector.tensor_tensor(
                        out=ntile[:, rc, :],
                        in0=trans_r,
                        in1=_ap(demc[:, rc, 0:1], [[0, L], [1, L]]),
                        op=OP.subtract,
                    )
                # patch sigma=0 (u=0) -> log-identity (host replays that step)
                nc.sync.dma_start(out=ntile[0:B, 0, :], in_=LOGID[:])

                def combine(a_ap, b_ap, out_ap, pcount, tagp):
                    t1 = crs.tile([128, 125], F32, tag="t1cmb")
                    mx = crs.tile([128, 25], F32, tag="mxcmb")
                    t2 = crs.tile([128, 125], F32, tag="t2cmb")
                    ex = crs.tile([128, 125], F32, tag="excmb")
                    se = crs.tile([128, 25], F32, tag="secmb")
                    lns = crs.tile([128, 25], F32, tag="lncmb")
                    pc = pcount
                    nc.vector.tensor_tensor(
                        out=t1[:pc, :],
                        in0=_ap(a_ap, [[5, L], [0, L], [1, L]]),
                        in1=_ap(b_ap, [[0, L], [1, L], [5, L]]),
                        op=OP.add,
                    )
                    nc.vector.reduce_max(
                        out=mx[:pc, :],
                        in_=_ap(t1[:pc, 0:1], [[5, 25], [1, 5]]),
                        axis=mybir.AxisListType.X,
                    )
                    nc.vector.tensor_tensor(
                        out=t2[:pc, :],
                        in0=t1[:pc, :],
                        in1=_ap(mx[:pc, 0:1], [[1, 25], [0, 5]]),
                        op=OP.subtract,
                    )
                    nc.scalar.activation(ex[:pc, :], t2[:pc, :], AF.Exp)
                    nc.vector.reduce_sum(
                        out=se[:pc, :],
                        in_=_ap(ex[:pc, 0:1], [[5, 25], [1, 5]]),
                        axis=mybir.AxisListType.X,
                    )
                    nc.scalar.activation(lns[:pc, :], se[:pc, :], AF.Ln)
                    nc.vector.tensor_tensor(
                        out=out_ap, in0=lns[:pc, :], in1=mx[:pc, :], op=OP.add
                    )

                cur = ntile
                nch_ = SBn
                lvl = 0
                while nch_ > 1:
                    nxt = crf.tile([128, nch_ // 2, 25], F32, tag=f"lv{lvl}")
                    for c in range(nch_ // 2):
                        combine(
                            cur[:, c, :], cur[:, c + nch_ // 2, :], nxt[:, c, :],
                            128, f"c{lvl}",
                        )
                    cur = nxt
                    nch_ //= 2
                    lvl += 1
                is3d = True
                pc = 64
                while pc >= B:
                    nxt = crf.tile([128, 25], F32, tag=f"pv{pc}")
                    if is3d:
                        a_ap, b_ap = cur[0:pc, 0, :], cur[pc:2 * pc, 0, :]
                    else:
                        a_ap, b_ap = cur[0:pc, :], cur[pc:2 * pc, :]
                    bt = crf.tile([64, 25], F32, tag="btcmb")
                    nc.sync.dma_start(out=bt[0:pc, :], in_=b_ap)
                    combine(a_ap, bt[0:pc, :], nxt[0:pc, :], pc, f"p{pc}")
                    cur = nxt
                    is3d = False
                    pc //= 2
                nc.sync.dma_start(out=OUT[:, 0:25], in_=cur[0:B, :])

                # ---- numerator partials (batched over all chunks) ----
                acc = crf.tile([128, SBn], F32, tag="acc")
                ohl = crs.tile([128, SBn, L], F32, tag="ohl")
                ohn = crs.tile([128, SBn, L], F32, tag="ohn")
                wexp = crs.tile([128, SBn, L, L], F32, tag="wexp")
                wred = crs.tile([128, SBn, L], F32, tag="wred")
                nc.vector.tensor_tensor(
                    out=ohl[:], in0=_ap(labc[:, 0:1], [[1, SBn], [0, L]]),
                    in1=_ap(iota_r[:, 0:1], [[0, SBn], [1, L]]), op=OP.is_equal,
                )
                nc.vector.tensor_tensor(
                    out=ohn[:], in0=_ap(labn[:, 0:1], [[1, SBn], [0, L]]),
                    in1=_ap(iota_r[:, 0:1], [[0, SBn], [1, L]]), op=OP.is_equal,
                )
                # wexp[rc, j, i] = ohl[rc, i] * trans[i, j]
                nc.vector.tensor_tensor(
                    out=wexp[:],
                    in0=_ap(ohl[:, 0, 0:1], [[5, SBn], [0, L], [1, L]]),
                    in1=_ap(trans_r[:, 0:1], [[0, SBn], [1, L], [5, L]]),
                    op=OP.mult,
                )
                nc.vector.reduce_sum(
                    out=wred[:], in_=_ap(wexp[:, 0, 0, 0:1], [[5, SBn * L], [1, L]]),
                    axis=mybir.AxisListType.X,
                )
                nc.vector.tensor_tensor(out=wred[:], in0=wred[:], in1=ohn[:], op=OP.mult)
                nc.vector.tensor_tensor(out=ohl[:], in0=demc[:], in1=ohl[:], op=OP.mult)
                nc.vector.tensor_tensor(out=wred[:], in0=wred[:], in1=ohl[:], op=OP.subtract)
                nc.vector.reduce_sum(
                    out=acc[:], in_=_ap(wred[:, 0, 0:1], [[5, SBn], [1, L]]),
                    axis=mybir.AxisListType.X,
                )
                psN = psC2.tile([B, SBn], F32, tag="ps")
                nc.tensor.matmul(psN[:], sel32[:], acc[:], start=True, stop=True)
                num32 = crs.tile([B, 1], F32, tag="num32")
                nc.vector.reduce_sum(out=num32[:], in_=psN[:], axis=mybir.AxisListType.X)
                nc.sync.dma_start(out=OUT[:, 25:26], in_=num32[:])

    return P


# ===========================================================================
# host side
# ===========================================================================


def _blocks_from_langs(langs):
    pi = np.argsort(langs, kind="stable")
    blocks = []
    j = 0
    while j < B:
        lg = int(langs[pi[j]])
        j1 = j
        while j1 < B and int(langs[pi[j1]]) == lg and j1 - j < IBMAX:
            j1 += 1
        blocks.append((lg, j, j1 - j))
        j = j1
    return pi, blocks


def _prep_core(inputs, core, pi):
    f = lambda a: np.asarray(a, np.float32)
    x = f(inputs["sequence_output"])
    langs = np.asarray(inputs["language_ids"]).astype(np.int64)
    labels = np.asarray(inputs["labels"]).astype(np.int64)
    aW1 = f(inputs["aW1"])
    alng = f(inputs["alng"])
    aW2 = f(inputs["aW2"])
    Wih_f, Wih_b = f(inputs["Wih_f"]), f(inputs["Wih_b"])
    Whh_f, Whh_b = f(inputs["Whh_f"]), f(inputs["Whh_b"])
    projW = f(inputs["projW"])
    pW1, plng = f(inputs["pW1"]), f(inputs["plng"])
    pW2 = f(inputs["pW2"])
    protos = f(inputs["prototypes"])
    sef = f(inputs["support_entity_features"])
    temp = float(np.asarray(inputs["temperature"]).reshape(-1)[0])
    trans = f(inputs["trans"])

    for nm in ("ab1", "alnb", "ab2", "b_f", "b_b", "projb", "pb1", "plnb", "pb2"):
        assert np.all(f(inputs[nm]) == 0.0), f"{nm} nonzero; not implemented"
    assert np.all(alng > 0.0)
    assert np.all(np.asarray(inputs["attention_mask"]) == 1)

    w, d = core >> 1, core & 1
    Wih, Whh = (Wih_f, Whh_f) if d == 0 else (Wih_b, Whh_b)

    # local step -> global t
    if d == 0:
        g_t = lambda s: 128 * w - WU + s
    else:
        g_t = lambda s: 128 * w + 127 + WU - s

    # gate block order (i,f,g,o) <- pytorch (i,f,g,o), pre-scaled for all-tanh
    src_off = {0: 0, 1: HL, 2: 2 * HL, 3: 3 * HL}
    gsc = {0: 0.5, 1: 0.5, 2: 1.0, 3: 0.5}
    perm = np.empty(4 * HL, np.int64)
    scale = np.empty(4 * HL, np.float32)
    for g_ in range(4):
        for u in range(HL):
            perm[g_ * HL + u] = src_off[g_] + u
            scale[g_ * HL + u] = gsc[g_]

    # XT: [128, NCH, 6, CH*B] in e3m4; chunk col r = j*CH + tc
    ts = np.array([g_t(s) for s in range(T)])
    valid = (ts >= 0) & (ts < S)
    xw = np.zeros((T, B, H), np.float32)
    xw[valid] = x[pi][:, ts[valid]].transpose(1, 0, 2)
    # xt[p, ch, k, j*CH+tc] = xw[ch*CH+tc, j, 128k+p]
    xt = np.ascontiguousarray(
        xw.reshape(NCH, CH, B, 6, 128).transpose(4, 0, 3, 2, 1)
    ).reshape(128, NCH, 6, B * CH)
    XTl = np.clip(xt, -15.5, 15.5).astype(NP8)

    # W1c: mean-centered W1 (per lang): y_c = y - mean_out(y) folds to
    # W1c[i,o] = W1[i,o] - rowmean(W1[i,:])
    w1t = np.empty((128, NLANG, 6, 6, 128), np.float32)
    wfh = np.empty((128, NLANG, 6, 8, 128), np.float32)
    for lg in range(NLANG):
        W1c = aW1[lg] - (aW1[lg].sum(axis=1, keepdims=True) / H)
        w1t[:, lg] = W1c.reshape(6, 128, 6, 128).transpose(1, 0, 2, 3)
        W2e = alng[lg][:, None] * aW2[lg]
        WF = W2e @ (Wih[:, perm] * scale[None, :])
        wfh[:, lg] = WF.reshape(6, 128, 8, 128).transpose(1, 0, 2, 3)
    W1Tl = np.clip(w1t, -15.5, 15.5).astype(NP8)
    WFhl = np.clip(wfh, -15.5, 15.5).astype(NP8)

    wh = Whh[:, perm] * (scale[None, :] * 0.5)
    WHHl = np.clip(
        wh.reshape(2, 128, 8, 128).transpose(1, 0, 2, 3), -15.5, 15.5
    ).astype(NP8)

    pjl = (0.5 * projW).reshape(2, 2, 128, EF).transpose(2, 0, 1, 3)
    pjl = np.ascontiguousarray(pjl).astype(NP16)
    # center pW1 so the proto-proj LN mean is exactly 0 on device
    pW1c = pW1 - pW1.mean(axis=1, keepdims=True)
    pw1l = pW1c.reshape(2, 128, PD).transpose(1, 0, 2).astype(NP16)
    pw2l = (plng[:, None] * pW2).astype(NP16)
    seftl = sef.T.reshape(2, 128, L).transpose(1, 0, 2).astype(NP16)
    protl = protos.T.astype(NP16)

    sel32 = np.zeros((128, B), np.float32)
    for p in range(128):
        sel32[p, p % B] = 1.0
    trr = np.broadcast_to(trans.reshape(1, 25), (128, 25)).copy()
    iotar = np.broadcast_to(np.arange(L, dtype=np.float32), (128, L)).copy()
    logid = np.full((B, 25), NEG, np.float32)
    logid[:, [0, 6, 12, 18, 24]] = 0.0

    labcc = np.zeros((128, SBn), np.float32)
    labnn = np.zeros((128, SBn), np.float32)
    for rc in range(SBn):
        for p in range(128):
            sigma = rc * 4 + p // B
            j = p % B
            u = _rev(sigma, 7)
            t = 128 * w + u
            it = pi[j]
            labcc[p, rc] = float(labels[it, t])
            labnn[p, rc] = float(labels[it, t + 1]) if u < 127 else 99.0

    return dict(
        XT=XTl, W1T=W1Tl, WFh=WFhl, WHH=WHHl,
        IDN8=np.eye(128, dtype=NP8), ONESW=np.ones((128, 128), NP16),
        PJh=pjl, PW1h=pw1l, PW2h=pw2l, SEFT=seftl, PROT=protl,
        IDN=np.eye(128, dtype=NP16), SEL32=sel32,
        ONES1=np.ones((128, 1), np.float32), TRR=trr, IOTA=iotar,
        LOGID=logid, LABC=labcc, LABN=labnn,
        TINV2=np.full((128, 1), 1.0 / (temp * temp), np.float32),
    )


def _lse(a, axis=-1):
    m = a.max(axis=axis, keepdims=True)
    return (np.log(np.exp(a - m).sum(axis=axis, keepdims=True)) + m).squeeze(axis)


def _host_combine(outs, inputs, pi):
    """outs: list of per-core OUT arrays [32, 32]."""
    labels = np.asarray(inputs["labels"]).astype(np.int64)
    start = np.asarray(inputs["start_trans"], np.float64)
    end = np.asarray(inputs["end_trans"], np.float64)
    trans = np.asarray(inputs["trans"], np.float64)

    # device item j -> original item pi[j]
    P = [outs[2 * w][:, 0:25].astype(np.float64).reshape(B, L, L) for w in range(4)]
    numw = [outs[2 * w][:, 25].astype(np.float64) for w in range(4)]
    em0 = [-outs[2 * w][:, 26:31].astype(np.float64) for w in range(4)]
    plv = outs[0][0:L, 31].astype(np.float64)

    labp = labels[pi]  # [32 device-j, 512]
    alpha = start[None, :] + em0[0]  # [32, 5]
    num = np.zeros(B)
    for w in range(4):
        if w > 0:
            alpha = _lse(alpha[:, :, None] + trans[None], axis=1) + em0[w]
            num += trans[labp[:, 128 * w - 1], labp[:, 128 * w]]
        alpha = _lse(alpha[:, :, None] + P[w], axis=1)
        num += numw[w]
    logZ = _lse(alpha + end[None, :], axis=1)
    num += start[labp[:, 0]] + end[labp[:, -1]]
    crf = -float((num - logZ).mean())
    pl = float(plv.sum()) / L
    return np.float32(crf + PROTO_W * pl)


_CACHED = {}


def _get_nc(blocks_key, blocks):
    if blocks_key not in _CACHED:
        nc = bacc.Bacc(None, target_bir_lowering=False)
        build_kernel(nc, blocks)
        nc.compile()
        _CACHED[blocks_key] = nc
    return _CACHED[blocks_key]


def kernel(**inputs) -> np.ndarray:
    langs = np.asarray(inputs["language_ids"]).astype(np.int64)
    pi, blocks = _blocks_from_langs(langs)
    nc = _get_nc(tuple(blocks), blocks)
    in_maps = [_prep_core(inputs, c, pi) for c in range(NCORES)]
    res = run_bass_kernel_spmd(nc, in_maps, list(range(NCORES)))
    outs = [res.results[c]["OUT"] for c in range(NCORES)]
    return _host_combine(outs, inputs, pi)

